# revision 12
# baseline (speedup 1.0000x reference)
"""Trainium2 Bass kernel for the 2-layer ChebConv (K=4) GNN with graph pooling.

Strategy (8 NeuronCores, SPMD single program):
  - Nodes sharded into 8 contiguous slabs by destination; each core owns the
    segmented-sum rows for its slab.  Edge structure preprocessed on host into
    (chunk x dest-block x fixed-128-dest-window) tiles of 128 edges.
  - Sparse L_hat @ h: u kept FEATURE-MAJOR everywhere.  Per source chunk the
    scaled u slab is loaded into a [128, CHUNK] fp32 SBUF table (2 copies);
    gpsimd ap_gather streams per-edge feature columns out of it (2 edge sets
    per instruction, one per 4-core group).  Messages are converted to fp16
    (scalar engine), XBAR DMA-transposed to edge-major [128, T, 128] tiles,
    then scatter-accumulated into PSUM dest windows by PE matmuls against
    on-chip one-hot matrices (fixed 128-wide windows -> no dynamic slices).
  - diag of L_hat is exactly 0 (2/lambda_max - 1), so L_hat@h is pure scatter.
  - w_e = -dis[row]*ew*dis[col]: dis[row] folded into the AllGathered u,
    ew into the one-hot values, -dis[col] (and the Chebyshev 2x) applied on
    the PSUM->accumulator path via a broadcast-row multiply.
  - deg = segment_sum(ew, row): one-hot scatter matmuls, ones stationary,
    fixed 64-wide windows.
  - u_k = dis*Tx_k AllGathered feature-major between rounds.
  - Pooling: h2 AllGathered feature-major; per-graph segment reduces with
    compile-time global graph boundaries; linear head on device.
"""

import math
import numpy as np

NC = 8
BLK = 512
TILE = 128
WSC = 128          # scatter window width
WDG = 64           # degree window width
G_FIXED = 256
HALF_CAP = 16      # max tiles per gather half (SBUF budget)


# ----------------------------------------------------------------------------
# Host-side preprocessing (integer / layout only -- no float arithmetic)
# ----------------------------------------------------------------------------

def _prep_scatter(row, col, ew, N, S, chunk):
    """Per-core structures for the scatter segmented sums.

    Tiles of <=128 edges; each tile targets one fixed WSC-wide dest window of
    one dest block, sources within one chunk.  Tile counts padded to the max
    across cores per (chunk, block, window) so the SPMD program is uniform.
    """
    nblk = math.ceil(S / BLK)
    nch = N // chunk
    nwin = BLK // WSC
    percore = []
    for i in range(NC):
        sel = np.nonzero((col // S) == i)[0]
        dloc = (col[sel] - i * S).astype(np.int64)
        ch = row[sel] // chunk
        b = dloc // BLK
        w = (dloc % BLK) // WSC
        key = ((ch * nblk + b) * nwin + w)
        order = np.argsort(key * (2 * N) + row[sel], kind="stable")
        sel = sel[order]
        cnt = np.bincount(key[order], minlength=nch * nblk * nwin)
        bound = np.concatenate([[0], np.cumsum(cnt)])
        percore.append((sel, bound))

    # padded tile counts per (ch, b, w)
    T = np.zeros((nch, nblk, nwin), np.int64)
    for i in range(NC):
        cnt = np.diff(percore[i][1]).reshape(nch, nblk, nwin)
        T = np.maximum(T, -(-cnt // TILE))
    blkw = [min(BLK, S - b * BLK) for b in range(nblk)]

    # cell = (ch, b); units split cells into gather groups of <=2*HALF_CAP tiles
    cells = []
    t0 = 0
    for ch in range(nch):
        for b in range(nblk):
            tc = int(T[ch, b].sum())
            if tc == 0:
                continue
            wins = []        # window id per tile, in tile order
            for w in range(nwin):
                wins += [w] * int(T[ch, b, w])
            units = []
            u0 = 0
            while u0 < tc:
                un = min(tc - u0, 2 * HALF_CAP)
                units.append((u0, un, -(-un // 2)))   # (start, ntiles, half)
                u0 += un
            cells.append(dict(ch=ch, b=b, t0=t0, tc=tc, wins=wins, units=units))
            t0 += tc
    TOT = t0

    idxcols = sum(u[2] * (TILE // 16) for c in cells for u in c["units"])

    out = []
    for i in range(NC):
        sel, bound = percore[i]
        drel = np.full((TILE, TOT), 255, np.uint8)
        ewv = np.zeros((TILE, TOT), np.float16)
        srel = np.zeros((TOT, TILE), np.int64)   # slot -> chunk-relative src
        for c in cells:
            ch, b = c["ch"], c["b"]
            t = c["t0"]
            for w in range(nwin):
                k = (ch * nblk + b) * nwin + w
                lo, hi = int(bound[k]), int(bound[k + 1])
                eids = sel[lo:hi]
                for tt in range(int(T[ch, b, w])):
                    e = eids[tt * TILE : (tt + 1) * TILE]
                    kk = len(e)
                    if kk:
                        drel[:kk, t] = (col[e] - i * S - b * BLK - w * WSC
                                        ).astype(np.uint8)
                        ewv[:kk, t] = ew[e].astype(np.float16)
                        srel[t, :kk] = row[e] - ch * chunk
                    t += 1

        # gather index stream: per unit, halves A (cores 0-3) / B (cores 4-7)
        idxw = np.zeros((128, idxcols), np.int16)
        c0 = 0
        for c in cells:
            for (u0, un, half) in c["units"]:
                na = half
                nb = un - na
                cols = half * (TILE // 16)
                for hs, hn, rows in ((0, na, range(0, 64)),
                                     (na, nb, range(64, 128))):
                    flat = np.zeros(half * TILE, np.int64)
                    for j in range(hn):
                        flat[j * TILE : (j + 1) * TILE] = \
                            srel[c["t0"] + u0 + hs + j]
                    wrap = np.zeros((16, cols), np.int16)
                    ssl = np.arange(half * TILE)
                    wrap[ssl % 16, ssl // 16] = flat.astype(np.int16)
                    for r0 in range(rows.start, rows.stop, 16):
                        idxw[r0 : r0 + 16, c0 : c0 + cols] = wrap
                c0 += cols
        out.append(dict(drel=drel, ewv=ewv, idxw=idxw))
    return out, cells, TOT, idxcols, blkw


def _prep_deg(row, ew, N, S):
    """Per-core row-sorted structures for the degree computation
    (fixed WDG-wide windows)."""
    nblk = math.ceil(S / BLK)
    nwin = BLK // WDG
    percore = []
    for i in range(NC):
        sel = np.nonzero((row // S) == i)[0]
        rloc = (row[sel] - i * S).astype(np.int64)
        key = (rloc // BLK) * nwin + (rloc % BLK) // WDG
        order = np.argsort(key, kind="stable")
        sel = sel[order]
        cnt = np.bincount(key[order], minlength=nblk * nwin)
        bound = np.concatenate([[0], np.cumsum(cnt)])
        percore.append((sel, bound))
    T2 = np.zeros((nblk, nwin), np.int64)
    for i in range(NC):
        cnt = np.diff(percore[i][1]).reshape(nblk, nwin)
        T2 = np.maximum(T2, -(-cnt // TILE))
    TOT2 = int(T2.sum())
    blkw = [min(BLK, S - b * BLK) for b in range(nblk)]
    out = []
    for i in range(NC):
        sel, bound = percore[i]
        drel = np.full((TILE, TOT2), 255, np.uint8)
        ewv = np.zeros((TILE, TOT2), np.float32)
        t = 0
        for b in range(nblk):
            for w in range(nwin):
                k = b * nwin + w
                lo, hi = int(bound[k]), int(bound[k + 1])
                eids = sel[lo:hi]
                for tt in range(int(T2[b, w])):
                    e = eids[tt * TILE : (tt + 1) * TILE]
                    kk = len(e)
                    if kk:
                        drel[:kk, t] = (row[e] - i * S - b * BLK - w * WDG
                                        ).astype(np.uint8)
                        ewv[:kk, t] = ew[e]
                    t += 1
        out.append(dict(drel=drel, ewv=ewv))
    return out, T2, TOT2, blkw


# ----------------------------------------------------------------------------
# Bass program
# ----------------------------------------------------------------------------

def _build(cfg):
    from concourse import bass, bacc, tile, mybir, library_config
    import contextlib

    f32 = mybir.dt.float32
    f16 = mybir.dt.float16
    i16 = mybir.dt.int16
    u8 = mybir.dt.uint8

    N, S, F, G = cfg["N"], cfg["S"], cfg["F"], cfg["G"]
    K = cfg["K"]
    CHUNK = cfg["CHUNK"]
    NCH = N // CHUNK
    cells = cfg["cells"]
    TOT, IDXCOLS = cfg["TOT"], cfg["IDXCOLS"]
    T2, TOT2 = cfg["T2"], cfg["TOT2"]
    blkw = cfg["blkw"]
    nblk = len(blkw)
    nwin2 = BLK // WDG
    gb = cfg["graph_bounds"]

    nc = bacc.Bacc("TRN2", target_bir_lowering=False, debug=False,
                   num_devices=NC)

    def din(name, shape, dt):
        return nc.dram_tensor(name, shape, dt, kind="ExternalInput")

    t_xfm = din("x_fm", [F, S], f32)
    t_xfm16 = din("x_fm16", [F, S], f16)
    t_idxw = din("idxw", [128, IDXCOLS], i16)
    t_drel = din("drel", [TILE, TOT], u8)
    t_ewv = din("ewv", [TILE, TOT], f16)
    t_ddrel = din("ddrel", [TILE, TOT2], u8)
    t_dewv = din("dewv", [TILE, TOT2], f32)
    t_w1 = din("w1", [F, K * F], f16)
    t_w2 = din("w2", [F, K * F], f16)
    t_b1 = din("b1c", [F, 1], f32)
    t_b2 = din("b2c", [F, 1], f32)
    t_gam = din("gam", [F, 1], f32)
    t_bet = din("bet", [F, 1], f32)
    t_mu = din("muv", [F, 1], f32)
    t_var = din("varv", [F, 1], f32)
    t_linw = din("linwt", [F, 6], f32)
    t_linb = din("linbc", [2, 1], f32)
    t_cnt = din("cntf", [1, G], f32)
    t_io128 = din("io128", [TILE, 8 * WSC], u8)
    t_io64 = din("io64", [TILE, 16 * WDG], u8)
    t_out = nc.dram_tensor("out", [2, G], f32, kind="ExternalOutput")

    rg = [list(range(NC))]

    with tile.TileContext(nc) as tc:
        ctx = contextlib.ExitStack()
        with ctx:
            sb = ctx.enter_context(tc.tile_pool(name="sb", bufs=1))
            ps = ctx.enter_context(tc.tile_pool(name="ps", bufs=1, space="PSUM"))
            dr = ctx.enter_context(tc.tile_pool(name="dr", bufs=1, space="DRAM"))

            nc.gpsimd.load_library(library_config.ap_gather)

            # ---------------- persistent loads ----------------
            io128 = sb.tile([TILE, 8 * WSC], u8)
            nc.sync.dma_start(out=io128[:], in_=t_io128[:, :])
            io64 = sb.tile([TILE, 16 * WDG], u8)
            nc.sync.dma_start(out=io64[:], in_=t_io64[:, :])
            ones_sb = sb.tile([TILE, 1], f32)
            nc.vector.memset(ones_sb[:], 1.0)
            ones1f = sb.tile([1, F], f32)
            nc.vector.memset(ones1f[:], 1.0)
            w1_sb = sb.tile([F, K * F], f16)
            nc.sync.dma_start(out=w1_sb[:], in_=t_w1[:, :])
            w2_sb = sb.tile([F, K * F], f16)
            nc.sync.dma_start(out=w2_sb[:], in_=t_w2[:, :])
            b1_sb = sb.tile([F, 1], f32)
            nc.sync.dma_start(out=b1_sb[:], in_=t_b1[:, :])
            b2_sb = sb.tile([F, 1], f32)
            nc.sync.dma_start(out=b2_sb[:], in_=t_b2[:, :])
            linw_sb = sb.tile([F, 6], f32)
            nc.sync.dma_start(out=linw_sb[:], in_=t_linw[:, :])
            linb_sb = sb.tile([2, 1], f32)
            nc.sync.dma_start(out=linb_sb[:], in_=t_linb[:, :])
            cnt_sb = sb.tile([1, G], f32)
            nc.sync.dma_start(out=cnt_sb[:], in_=t_cnt[:, :])

            gam_sb = sb.tile([F, 1], f32)
            nc.sync.dma_start(out=gam_sb[:], in_=t_gam[:, :])
            bet_sb = sb.tile([F, 1], f32)
            nc.sync.dma_start(out=bet_sb[:], in_=t_bet[:, :])
            mu_sb = sb.tile([F, 1], f32)
            nc.sync.dma_start(out=mu_sb[:], in_=t_mu[:, :])
            var_sb = sb.tile([F, 1], f32)
            nc.sync.dma_start(out=var_sb[:], in_=t_var[:, :])
            bnscale = sb.tile([F, 1], f32)
            bnbias = sb.tile([F, 1], f32)
            tmp1 = sb.tile([F, 1], f32)
            nc.vector.tensor_scalar_add(tmp1[:], var_sb[:], 1e-5)
            nc.vector.reciprocal(tmp1[:], tmp1[:])
            nc.scalar.sqrt(tmp1[:], tmp1[:])
            nc.vector.tensor_mul(bnscale[:], gam_sb[:], tmp1[:])
            nc.vector.tensor_mul(tmp1[:], bnscale[:], mu_sb[:])
            nc.vector.tensor_sub(bnbias[:], bet_sb[:], tmp1[:])

            dis_dram = dr.tile([1, S], f32)
            dism1_dram = dr.tile([1, S], f32)
            dism2_dram = dr.tile([1, S], f32)
            h1_dram = dr.tile([F, S], f16)
            tx1_dram = dr.tile([F, S], f16)
            oacc_dram = dr.tile([F, S], f32)
            u_full = [dr.tile([NC * F, S], f32, addr_space="Shared",
                              name=f"u_full{r}") for r in range(6)]
            h2_full = dr.tile([NC * F, S], f32, addr_space="Shared",
                              name="h2_full")

            # big persistent SBUF tensors
            y_acc = sb.tile([F, S], f16, name="y_acc")
            table = sb.tile([128, CHUNK], f32, name="utable")

            # ---------------- helpers ----------------
            def build_onehot(oh, drel_t, ew_t, Tb, W, iot, dt):
                nc.vector.tensor_tensor(
                    out=oh[:, : Tb * W],
                    in0=iot[:, : Tb * W],
                    in1=drel_t.unsqueeze(-1).to_broadcast([TILE, Tb, W]),
                    op=mybir.AluOpType.is_equal)
                nc.vector.tensor_tensor(
                    out=oh[:, : Tb * W],
                    in0=oh[:, : Tb * W],
                    in1=ew_t.unsqueeze(-1).to_broadcast([TILE, Tb, W]),
                    op=mybir.AluOpType.mult)

            # ---------------- deg phase ----------------
            t0 = 0
            for b in range(nblk):
                bw = blkw[b]
                dps = ps.tile([F, BLK], f32, tag="cps", bufs=2)
                nc.vector.memset(dps[0:1, :], 0.0)
                Tball = int(T2[b].sum())
                done = 0
                for w in range(nwin2):
                    Tw = int(T2[b][w])
                    for gs in range(0, Tw, 8):
                        gn = min(8, Tw - gs)
                        tg = t0 + done + gs
                        ddrel_t = sb.tile([TILE, 8], u8, tag="ddrel", bufs=2)
                        nc.sync.dma_start(out=ddrel_t[:, :gn],
                                          in_=t_ddrel[:, tg : tg + gn])
                        dewv_t = sb.tile([TILE, 8], f32, tag="dewv", bufs=2)
                        nc.sync.dma_start(out=dewv_t[:, :gn],
                                          in_=t_dewv[:, tg : tg + gn])
                        doh = sb.tile([TILE, 8 * WDG], f32, tag="doh", bufs=2)
                        build_onehot(doh, ddrel_t[:, :gn], dewv_t[:, :gn],
                                     gn, WDG, io64, f32)
                        for j in range(gn):
                            t = done + gs + j
                            nc.tensor.matmul(
                                out=dps[0:1, w * WDG : (w + 1) * WDG],
                                lhsT=ones_sb[:],
                                rhs=doh[:, j * WDG : (j + 1) * WDG],
                                start=False, stop=(t == Tball - 1),
                                skip_group_check=True)
                    done += Tw
                drow = sb.tile([1, BLK], f32, tag="drow", bufs=2)
                mrow = sb.tile([1, BLK], f32, tag="mrow", bufs=2)
                nc.vector.tensor_scalar(
                    out=mrow[0:1, :bw], in0=dps[0:1, :bw], scalar1=0.0,
                    scalar2=None, op0=mybir.AluOpType.is_gt)
                nc.vector.tensor_scalar_max(drow[0:1, :bw], dps[0:1, :bw], 1e-30)
                nc.vector.reciprocal(drow[0:1, :bw], drow[0:1, :bw])
                nc.scalar.sqrt(drow[0:1, :bw], drow[0:1, :bw])
                nc.vector.tensor_mul(drow[0:1, :bw], drow[0:1, :bw], mrow[0:1, :bw])
                nc.sync.dma_start(out=dis_dram[0:1, b * BLK : b * BLK + bw],
                                  in_=drow[0:1, :bw])
                nc.vector.tensor_scalar_mul(mrow[0:1, :bw], drow[0:1, :bw], -1.0)
                nc.sync.dma_start(out=dism1_dram[0:1, b * BLK : b * BLK + bw],
                                  in_=mrow[0:1, :bw])
                nc.vector.tensor_scalar_mul(mrow[0:1, :bw], drow[0:1, :bw], -2.0)
                nc.sync.dma_start(out=dism2_dram[0:1, b * BLK : b * BLK + bw],
                                  in_=mrow[0:1, :bw])
                t0 += Tball

            # ---------------- round helpers ----------------
            def rep_row(src_dram, b, bw):
                """PSUM [F, bw] broadcast of a DRAM row slice."""
                dm_t = sb.tile([1, BLK], f32, tag="dm_t", bufs=2)
                nc.sync.dma_start(out=dm_t[0:1, :bw],
                                  in_=src_dram[0:1, b * BLK : b * BLK + bw])
                rep = ps.tile([F, BLK], f32, tag="rep", bufs=1)
                nc.tensor.matmul(out=rep[:F, :bw], lhsT=ones1f[:],
                                 rhs=dm_t[0:1, :bw], start=True, stop=True)
                return rep

            def scale_to_u_and_ag(use_yacc, rnd):
                """u = dis * tx, feature-major, AllGather to u_full[rnd]."""
                ag_in = dr.tile([F, S], f32, tag="ag_in", bufs=2,
                                name=f"agin{rnd}")
                for b in range(nblk):
                    bw = blkw[b]
                    rep = rep_row(dis_dram, b, bw)
                    if use_yacc:
                        srcap = y_acc[:, b * BLK : b * BLK + bw]
                    else:
                        st = sb.tile([F, BLK], f32, tag="ust", bufs=2)
                        nc.sync.dma_start(out=st[:, :bw],
                                          in_=t_xfm[:, b * BLK : b * BLK + bw])
                        srcap = st[:, :bw]
                    stg = sb.tile([F, BLK], f32, tag="stg", bufs=2)
                    nc.vector.tensor_tensor(out=stg[:, :bw], in0=srcap,
                                            in1=rep[:F, :bw],
                                            op=mybir.AluOpType.mult)
                    nc.sync.dma_start(out=ag_in[:, b * BLK : b * BLK + bw],
                                      in_=stg[:, :bw])
                nc.gpsimd.collective_compute(
                    "AllGather", mybir.AluOpType.bypass, replica_groups=rg,
                    ins=[ag_in[:]], outs=[u_full[rnd][:, :]])

            def seg_sum_round(rnd, dism_tag):
                """y_acc = (L_hat @ u) scaled; fp16 [F, S]."""
                nc.vector.memset(y_acc[:], 0.0)
                ic0 = [0]
                for ch in range(NCH):
                    # load scaled-u chunk table (2 copies for the 2 edge sets)
                    for cp in range(2):
                        for half in range(2):
                            g0 = (ch * 2 + half) * F
                            nc.sync.dma_start(
                                out=table[cp * F : (cp + 1) * F,
                                          half * S : (half + 1) * S],
                                in_=u_full[rnd][g0 : g0 + F, :])
                    for c in cells:
                        if c["ch"] != ch:
                            continue
                        b = c["b"]
                        bw = blkw[b]
                        sps = ps.tile([F, BLK], f32, tag="sps", bufs=2)
                        nc.vector.memset(sps[:], 0.0)
                        tc_ = c["tc"]
                        wins = c["wins"]
                        mm = 0
                        for (u0, un, half) in c["units"]:
                            nidx = half * TILE
                            cols = half * (TILE // 16)
                            idxt = sb.tile([128, HALF_CAP * 8], i16,
                                           tag="idxt", bufs=2)
                            nc.sync.dma_start(
                                out=idxt[:, :cols],
                                in_=t_idxw[:, ic0[0] : ic0[0] + cols])
                            ic0[0] += cols
                            mgT = sb.tile([128, HALF_CAP * TILE], f32,
                                          tag="mgT", bufs=2)
                            nc.gpsimd.ap_gather(
                                out_ap=mgT[:, :nidx],
                                in_ap=table[:, :],
                                idxs_ap=idxt[:, :cols],
                                channels=128, num_elems=CHUNK, d=1,
                                num_idxs=nidx)
                            mgT16 = sb.tile([128, HALF_CAP * TILE], f16,
                                            tag="mgT16", bufs=2)
                            nc.scalar.activation(
                                out=mgT16[:, :nidx], in_=mgT[:, :nidx],
                                func=mybir.ActivationFunctionType.Identity)
                            ms = sb.tile([128, HALF_CAP, TILE], f16,
                                         tag="ms", bufs=2)
                            nc.sync.dma_start_transpose(
                                out=ms[:, :half, :], in_=mgT16[:, :nidx])
                            for gs in range(0, un, 8):
                                gn = min(8, un - gs)
                                tg = c["t0"] + u0 + gs
                                drel_t = sb.tile([TILE, 8], u8,
                                                 tag="drel", bufs=2)
                                nc.sync.dma_start(out=drel_t[:, :gn],
                                                  in_=t_drel[:, tg : tg + gn])
                                ewv_t = sb.tile([TILE, 8], f16,
                                                tag="ewv", bufs=2)
                                nc.sync.dma_start(out=ewv_t[:, :gn],
                                                  in_=t_ewv[:, tg : tg + gn])
                                oh = sb.tile([TILE, 8 * WSC], f16,
                                             tag="oh", bufs=2)
                                build_onehot(oh, drel_t[:, :gn], ewv_t[:, :gn],
                                             gn, WSC, io128, f16)
                                for j in range(gn):
                                    t = u0 + gs + j
                                    jl = gs + j
                                    w = wins[t]
                                    if jl < half:
                                        lhs = ms[:, jl, 0:F]
                                    else:
                                        lhs = ms[:, jl - half, F : 2 * F]
                                    mm += 1
                                    nc.tensor.matmul(
                                        out=sps[:F, w * WSC : (w + 1) * WSC],
                                        lhsT=lhs,
                                        rhs=oh[:, j * WSC : (j + 1) * WSC],
                                        start=False, stop=(mm == tc_),
                                        skip_group_check=True)
                        # accumulate into y_acc (fp16)
                        nc.vector.tensor_add(
                            y_acc[:, b * BLK : b * BLK + bw],
                            y_acc[:, b * BLK : b * BLK + bw],
                            sps[:F, :bw])
                # scale by -dis (and 2x for higher orders)
                src = dism1_dram if dism_tag == 1 else dism2_dram
                for b in range(nblk):
                    bw = blkw[b]
                    rep = rep_row(src, b, bw)
                    nc.vector.tensor_tensor(
                        out=y_acc[:, b * BLK : b * BLK + bw],
                        in0=y_acc[:, b * BLK : b * BLK + bw],
                        in1=rep[:F, :bw],
                        op=mybir.AluOpType.mult)

            def conv_accum(w_sb, k, first, src_dram=None):
                for b in range(nblk):
                    bw = blkw[b]
                    if src_dram is not None:
                        rhs_t = sb.tile([F, BLK], f16, tag="crhs", bufs=2)
                        nc.sync.dma_start(out=rhs_t[:, :bw],
                                          in_=src_dram[:, b * BLK : b * BLK + bw])
                        rhs = rhs_t[:, :bw]
                    else:
                        rhs = y_acc[:, b * BLK : b * BLK + bw]
                    cps = ps.tile([F, BLK], f32, tag="cps", bufs=2)
                    nc.tensor.matmul(
                        out=cps[:F, :bw],
                        lhsT=w_sb[:, k * F : (k + 1) * F],
                        rhs=rhs,
                        start=True, stop=True)
                    st = sb.tile([F, BLK], f32, tag="cst", bufs=2)
                    if first:
                        nc.vector.tensor_copy(st[:, :bw], cps[:F, :bw])
                    else:
                        nc.sync.dma_start(out=st[:, :bw],
                                          in_=oacc_dram[:, b * BLK : b * BLK + bw])
                        nc.vector.tensor_add(st[:, :bw], st[:, :bw], cps[:F, :bw])
                    nc.sync.dma_start(out=oacc_dram[:, b * BLK : b * BLK + bw],
                                      in_=st[:, :bw])

            def sub_dram(src_dram):
                for b in range(nblk):
                    bw = blkw[b]
                    st = sb.tile([F, BLK], f16, tag="cst16", bufs=2)
                    nc.sync.dma_start(out=st[:, :bw],
                                      in_=src_dram[:, b * BLK : b * BLK + bw])
                    nc.vector.tensor_sub(y_acc[:, b * BLK : b * BLK + bw],
                                         y_acc[:, b * BLK : b * BLK + bw],
                                         st[:, :bw])

            def save_yacc(dst):
                nc.sync.dma_start(out=dst[:, :], in_=y_acc[:])

            # ---------------- layer 1 ----------------
            scale_to_u_and_ag(False, 0)               # u0 = dis*x
            conv_accum(w1_sb, 0, True, src_dram=t_xfm16)

            seg_sum_round(0, 1)                       # y_acc = Tx1
            save_yacc(tx1_dram)
            conv_accum(w1_sb, 1, False)
            scale_to_u_and_ag(True, 1)

            seg_sum_round(1, 2)                       # y_acc = 2 L Tx1
            sub_dram(t_xfm16)                         # Tx2
            conv_accum(w1_sb, 2, False)
            scale_to_u_and_ag(True, 2)

            seg_sum_round(2, 2)
            sub_dram(tx1_dram)                        # Tx3
            conv_accum(w1_sb, 3, False)

            # h1 = bn(relu(oacc + b1))
            for b in range(nblk):
                bw = blkw[b]
                st = sb.tile([F, BLK], f32, tag="cst", bufs=2)
                nc.sync.dma_start(out=st[:, :bw],
                                  in_=oacc_dram[:, b * BLK : b * BLK + bw])
                nc.scalar.activation(
                    out=st[:, :bw], in_=st[:, :bw],
                    func=mybir.ActivationFunctionType.Relu,
                    bias=b1_sb[:, 0:1], scale=1.0)
                nc.scalar.activation(
                    out=y_acc[:, b * BLK : b * BLK + bw], in_=st[:, :bw],
                    func=mybir.ActivationFunctionType.Identity,
                    bias=bnbias[:, 0:1], scale=bnscale[:, 0:1])
            save_yacc(h1_dram)
            scale_to_u_and_ag(True, 3)
            conv_accum(w2_sb, 0, True)

            # ---------------- layer 2 ----------------
            seg_sum_round(3, 1)
            save_yacc(tx1_dram)
            conv_accum(w2_sb, 1, False)
            scale_to_u_and_ag(True, 4)

            seg_sum_round(4, 2)
            sub_dram(h1_dram)
            conv_accum(w2_sb, 2, False)
            scale_to_u_and_ag(True, 5)

            seg_sum_round(5, 2)
            sub_dram(tx1_dram)
            conv_accum(w2_sb, 3, False)

            ag2_in = dr.tile([F, S], f32)
            for b in range(nblk):
                bw = blkw[b]
                st = sb.tile([F, BLK], f32, tag="cst", bufs=2)
                nc.sync.dma_start(out=st[:, :bw],
                                  in_=oacc_dram[:, b * BLK : b * BLK + bw])
                nc.scalar.activation(
                    out=st[:, :bw], in_=st[:, :bw],
                    func=mybir.ActivationFunctionType.Relu,
                    bias=b2_sb[:, 0:1], scale=1.0)
                nc.sync.dma_start(out=ag2_in[:, b * BLK : b * BLK + bw],
                                  in_=st[:, :bw])
            nc.gpsimd.collective_compute(
                "AllGather", mybir.AluOpType.bypass, replica_groups=rg,
                ins=[ag2_in[:]], outs=[h2_full[:, :]])

            # ---------------- pooling ----------------
            s_cols = sb.tile([F, G], f32)
            mx_cols = sb.tile([F, G], f32)
            nc.vector.memset(s_cols[:], 0.0)
            nc.vector.memset(mx_cols[:], -1e30)
            t_acc = sb.tile([F, 1], f32)
            t_m = sb.tile([F, 1], f32)
            for c in range(NC):
                # reuse the (now idle) gather table SBUF space for pooling
                nc.sync.dma_start(out=table[0:F, 0:S],
                                  in_=h2_full[c * F : (c + 1) * F, :])
                lo_n, hi_n = c * S, (c + 1) * S
                g_lo = max(int(np.searchsorted(gb, lo_n, side="right")) - 1, 0)
                for g in range(g_lo, G):
                    if int(gb[g]) >= hi_n:
                        break
                    a = max(int(gb[g]), lo_n)
                    b_ = min(int(gb[g + 1]), hi_n)
                    if a >= b_:
                        continue
                    al, bl = a - lo_n, b_ - lo_n
                    whole = int(gb[g]) >= lo_n and int(gb[g + 1]) <= hi_n
                    if whole:
                        nc.vector.tensor_reduce(
                            out=s_cols[:, g : g + 1], in_=table[0:F, al:bl],
                            axis=mybir.AxisListType.X, op=mybir.AluOpType.add)
                        nc.vector.tensor_reduce(
                            out=mx_cols[:, g : g + 1], in_=table[0:F, al:bl],
                            axis=mybir.AxisListType.X, op=mybir.AluOpType.max)
                    else:
                        nc.vector.tensor_reduce(
                            out=t_acc[:, 0:1], in_=table[0:F, al:bl],
                            axis=mybir.AxisListType.X, op=mybir.AluOpType.add)
                        nc.vector.tensor_add(s_cols[:, g : g + 1],
                                             s_cols[:, g : g + 1], t_acc[:, 0:1])
                        nc.vector.tensor_reduce(
                            out=t_m[:, 0:1], in_=table[0:F, al:bl],
                            axis=mybir.AxisListType.X, op=mybir.AluOpType.max)
                        nc.vector.tensor_tensor(
                            out=mx_cols[:, g : g + 1], in0=mx_cols[:, g : g + 1],
                            in1=t_m[:, 0:1], op=mybir.AluOpType.max)

            rc = sb.tile([1, G], f32)
            nc.vector.tensor_scalar_max(rc[:], cnt_sb[:], 1.0)
            nc.vector.reciprocal(rc[:], rc[:])
            mean_cols = sb.tile([F, G], f32)
            rep2 = ps.tile([F, G], f32, tag="repg", bufs=1)
            nc.tensor.matmul(out=rep2[:F, :G], lhsT=ones1f[:],
                             rhs=rc[0:1, :], start=True, stop=True)
            nc.vector.tensor_tensor(out=mean_cols[:], in0=s_cols[:],
                                    in1=rep2[:F, :G], op=mybir.AluOpType.mult)
            mk = sb.tile([1, G], f32)
            nc.vector.tensor_scalar(out=mk[:], in0=cnt_sb[:], scalar1=0.0,
                                    scalar2=None, op0=mybir.AluOpType.is_gt)
            rep3 = ps.tile([F, G], f32, tag="repg", bufs=1)
            nc.tensor.matmul(out=rep3[:F, :G], lhsT=ones1f[:],
                             rhs=mk[0:1, :], start=True, stop=True)
            nc.vector.tensor_tensor(out=mx_cols[:], in0=mx_cols[:],
                                    in1=rep3[:F, :G], op=mybir.AluOpType.mult)

            hps = ps.tile([2, G], f32, tag="hps")
            for ci, pc in enumerate([s_cols, mean_cols, mx_cols]):
                nc.tensor.matmul(out=hps[:2, :G],
                                 lhsT=linw_sb[:, 2 * ci : 2 * ci + 2],
                                 rhs=pc[:],
                                 start=(ci == 0), stop=(ci == 2))
            outsb = sb.tile([2, G], f32)
            nc.scalar.activation(out=outsb[:], in_=hps[:2, :G],
                                 func=mybir.ActivationFunctionType.Identity,
                                 bias=linb_sb[:, 0:1], scale=1.0)
            nc.sync.dma_start(out=t_out[:, :], in_=outsb[:])

    nc.compile()
    return nc


# ----------------------------------------------------------------------------
# Entry point
# ----------------------------------------------------------------------------

def _run(x, edge_index, edge_weight, batch, W1, b1, bn_gamma, bn_beta,
         bn_mean, bn_var, W2, b2, linW, linb, G):
    from concourse.bass_utils import run_bass_kernel_spmd

    x = np.asarray(x)
    edge_index = np.asarray(edge_index)
    ew = np.asarray(edge_weight, dtype=np.float32)
    batch = np.asarray(batch)
    N, F = x.shape
    K = int(np.asarray(W1).shape[0])
    S = N // NC
    CHUNK = N // 4

    row = edge_index[0].astype(np.int64)
    col = edge_index[1].astype(np.int64)

    eprep, cells, TOT, IDXCOLS, blkw = _prep_scatter(row, col, ew, N, S, CHUNK)
    dprep, T2, TOT2, _ = _prep_deg(row, ew, N, S)
    gb = np.searchsorted(batch, np.arange(G + 1))
    cnt = (gb[1:] - gb[:-1]).astype(np.float32)

    io128 = np.tile((np.arange(8 * WSC) % WSC).astype(np.uint8), (TILE, 1))
    io64 = np.tile((np.arange(16 * WDG) % WDG).astype(np.uint8), (TILE, 1))

    cfg = dict(N=N, S=S, F=F, G=G, K=K, CHUNK=CHUNK,
               cells=cells, TOT=TOT, IDXCOLS=IDXCOLS,
               T2=T2, TOT2=TOT2, blkw=blkw, graph_bounds=gb)
    nc = _build(cfg)

    W1a = np.asarray(W1, np.float32)
    W2a = np.asarray(W2, np.float32)
    w1in = np.ascontiguousarray(W1a.transpose(1, 0, 2).reshape(F, K * F)
                                ).astype(np.float16)
    w2in = np.ascontiguousarray(W2a.transpose(1, 0, 2).reshape(F, K * F)
                                ).astype(np.float16)
    linWa = np.asarray(linW, np.float32)
    linwt = np.concatenate([linWa[:, F * c : F * (c + 1)].T
                            for c in range(3)], axis=1)

    in_maps = []
    for i in range(NC):
        ep, dp = eprep[i], dprep[i]
        in_maps.append({
            "x_fm": np.ascontiguousarray(x[i * S : (i + 1) * S].T.astype(np.float32)),
            "x_fm16": np.ascontiguousarray(x[i * S : (i + 1) * S].T.astype(np.float16)),
            "idxw": ep["idxw"],
            "drel": ep["drel"],
            "ewv": ep["ewv"],
            "ddrel": dp["drel"],
            "dewv": dp["ewv"],
            "w1": w1in, "w2": w2in,
            "b1c": np.asarray(b1, np.float32).reshape(F, 1),
            "b2c": np.asarray(b2, np.float32).reshape(F, 1),
            "gam": np.asarray(bn_gamma, np.float32).reshape(F, 1),
            "bet": np.asarray(bn_beta, np.float32).reshape(F, 1),
            "muv": np.asarray(bn_mean, np.float32).reshape(F, 1),
            "varv": np.asarray(bn_var, np.float32).reshape(F, 1),
            "linwt": np.ascontiguousarray(linwt),
            "linbc": np.asarray(linb, np.float32).reshape(2, 1),
            "cntf": cnt.reshape(1, G),
            "io128": io128,
            "io64": io64,
        })

    res = run_bass_kernel_spmd(nc, in_maps, core_ids=list(range(NC)))
    out = res.results[0]["out"]
    return np.ascontiguousarray(out.T)


def kernel(x, edge_index, edge_weight, batch, W1, b1, bn_gamma, bn_beta,
           bn_mean, bn_var, W2, b2, linW, linb):
    return _run(x, edge_index, edge_weight, batch, W1, b1, bn_gamma, bn_beta,
                bn_mean, bn_var, W2, b2, linW, linb, G_FIXED)
